# revision 1
# baseline (speedup 1.0000x reference)
# Multi-head attention on 8 Trainium2 NeuronCores — data-parallel over batch.
# v8: ring-3 sT PSUM (exp spine back-to-back ~1005-1113ns), par-outer scores,
#     v pass interleaved with av(0,0)/sc(0,1) per the 8-slot pexp rotation,
#     qk tiles injected at g1 so their DVE evictions clear before the
#     qc-end normalize chain, k/q weight blocks loaded first, w_proj last.
import sys
import types

import numpy as np


def _install_axon_profile_hook():
    try:
        import antenv.axon_hooks  # noqa: F401
        return
    except ImportError:
        pass
    try:
        import antenv
        from trn_agent_boot.trn_boot import _ntff_profile_via_ctypes

        m = types.ModuleType("antenv.axon_hooks")
        hook = _ntff_profile_via_ctypes("/opt/axon/libaxon_pjrt.so")
        m.get_axon_ntff_profile_hook = lambda: hook
        m.set_axon_ntff_profile_hook = lambda h: None
        antenv.axon_hooks = m
        sys.modules["antenv.axon_hooks"] = m
    except Exception:
        pass


N, C, H, D = 1024, 768, 12, 64
SCALE = D ** -0.5
NT = N // 128        # 8 token tiles
CT = C // 128        # 6 channel tiles
NQC = N // 512       # 2 q-chunks
E = D + 1            # per-head v width with ones column
NG = NT // 2         # 4 k-tile pair groups


def build_kernel():
    import concourse.bass as bass  # noqa: F401
    import concourse.mybir as mybir
    from concourse import bacc
    from concourse.tile import TileContext
    from concourse.masks import make_identity
    from contextlib import ExitStack

    F32 = mybir.dt.float32
    BF16 = mybir.dt.bfloat16
    Exp = mybir.ActivationFunctionType.Exp

    nc = bacc.Bacc()
    x_ext = nc.declare_dram_parameter("x", [N, C], F32, isOutput=False)
    wqkv_ext = nc.declare_dram_parameter("w_qkv", [C, 3 * C], F32, isOutput=False)
    wproj_ext = nc.declare_dram_parameter("w_proj", [C, C], F32, isOutput=False)
    bproj_ext = nc.declare_dram_parameter("b_proj", [C], F32, isOutput=False)
    out_ext = nc.declare_dram_parameter("out", [N, C], F32, isOutput=True)

    with TileContext(nc) as tc, ExitStack() as ctx:
        const = ctx.enter_context(tc.tile_pool(name="const", bufs=1))
        persist = ctx.enter_context(tc.tile_pool(name="persist", bufs=1))
        stage = ctx.enter_context(tc.tile_pool(name="stage", bufs=2))
        psum_sT = ctx.enter_context(tc.tile_pool(name="psum_sT", bufs=2, space="PSUM"))
        psum_av = ctx.enter_context(tc.tile_pool(name="psum_av", bufs=3, space="PSUM"))
        psum_mm = ctx.enter_context(tc.tile_pool(name="psum_mm", bufs=1, space="PSUM"))

        ident = const.tile([128, 128], BF16, tag="ident")
        make_identity(nc, ident)
        bf32 = const.tile([1, C], F32, tag="bf32")
        nc.sync.dma_start(out=bf32[:], in_=bproj_ext[None, :])
        b_bcast = const.tile([128, C], F32, tag="b_bcast")
        nc.gpsimd.partition_broadcast(b_bcast[:], bf32[:])

        w_bf = [persist.tile([128, 3 * C], BF16, tag=f"wbf{k}", name=f"wbf{k}")
                for k in range(CT)]
        wp_bf = [persist.tile([128, C], BF16, tag=f"wpbf{k}", name=f"wpbf{k}")
                 for k in range(CT)]
        xT = [persist.tile([128, N], BF16, tag=f"xT{c}", name=f"xT{c}")
              for c in range(CT)]
        qkT = [persist.tile([128, N], BF16, tag=f"qkT{m}", name=f"qkT{m}")
               for m in range(2 * CT)]
        v_aug = [persist.tile([128, H * E], BF16, tag=f"vaug{m}", name=f"vaug{m}")
                 for m in range(NT)]
        outT = [persist.tile([128, N], BF16, tag=f"outT{c}", name=f"outT{c}")
                for c in range(CT)]

        # ---- load x, one high-priority DMA per 128-token tile ----
        xpool_cm = tc.tile_pool(name="xpool", bufs=1)
        xpool = xpool_cm.__enter__()
        xall = xpool.tile([128, NT * C], F32, tag="xall", name="xall")
        with tc.high_priority():
            for t in range(NT):
                nc.sync.dma_start(
                    out=xall[:, t * C:(t + 1) * C],
                    in_=x_ext.rearrange("(t p) c -> p t c", p=128)[:, t, :])
        for t in range(NT):
            xbf = stage.tile([128, C], BF16, tag="xbf", name=f"xbf{t}")
            nc.vector.tensor_copy(xbf[:], xall[:, t * C:(t + 1) * C])
            for c in range(CT):
                trp = psum_av.tile([128, 128], BF16, tag="av", name=f"trp{t}_{c}")
                nc.tensor.transpose(trp[:], xbf[:, c * 128:(c + 1) * 128], ident[:])
                nc.scalar.copy(xT[c][:, t * 128:(t + 1) * 128], trp[:])
        xpool_cm.__exit__(None, None, None)
        expp = ctx.enter_context(tc.tile_pool(name="expp", bufs=8))
        rbp = ctx.enter_context(tc.tile_pool(name="rbp", bufs=2))

        # ---- load w_qkv by column blocks (k, q first — they gate the spine) ----
        wq_blocks = [(768, 512), (1280, 256), (0, 512), (512, 256),
                     (1536, 512), (2048, 256)]
        for bi, (cs, cw) in enumerate(wq_blocks):
            wcb = stage.tile([128, CT * 512], F32, tag="wcb", name=f"wcb{bi}")
            src = wqkv_ext.rearrange("(k p) c -> p k c", p=128)[:, :, cs:cs + cw]
            nc.sync.dma_start(out=wcb[:, :CT * cw].rearrange("p (k c) -> p k c", k=CT),
                              in_=src)
            for k in range(CT):
                nc.vector.tensor_copy(w_bf[k][:, cs:cs + cw],
                                      wcb[:, k * cw:(k + 1) * cw])

        def v_half(m, n):
            # half of v[m] = x[m] @ w_qkv[:,1536:]; alternates between the
            # 1-bank pool and the av ring (idle until step 0) so the matmul
            # chain is never paced by a single bank's eviction
            cs, cw = (1536, 512) if n == 0 else (2048, 256)
            va = v_aug[m].rearrange("p (h e) -> p h e", e=E)
            if n == 0:
                nc.vector.memset(va[:, :, D:E], 1.0)
            if m % 2 == 0:
                vps = psum_mm.tile([128, 512], F32, tag="mm",
                                   name=f"vps{m}_{n}")
            else:
                vps = psum_av.tile([128, 512], F32, tag="av",
                                   name=f"vps{m}_{n}")
            for kt in range(CT):
                nc.tensor.matmul(vps[:, :cw],
                                 xT[kt][:, m * 128:(m + 1) * 128],
                                 w_bf[kt][:, cs:cs + cw],
                                 start=(kt == 0), stop=(kt == CT - 1))
            nh = cw // D
            nc.vector.tensor_copy(
                va[:, n * 8:n * 8 + nh, 0:D],
                vps[:, :cw].rearrange("p (h e) -> p h e", e=D))

        def qk_half(m, n):
            # one 512-col half of a k/q tile via the 1-bank pool: 6 matmuls
            # + immediate short eviction
            qps = psum_mm.tile([128, 512], F32, tag="mm", name=f"qps{m}_{n}")
            for kt in range(CT):
                nc.tensor.matmul(qps[:],
                                 w_bf[kt][:, m * 128:(m + 1) * 128],
                                 xT[kt][:, n * 512:(n + 1) * 512],
                                 start=(kt == 0), stop=(kt == CT - 1))
            nc.vector.tensor_copy(qkT[m][:, n * 512:(n + 1) * 512], qps[:])

        def qk_tile(m):
            qk_half(m, 0)
            qk_half(m, 1)

        def sc_group(hp, qc, g, pexps):
            # scores + exp for one k-tile pair group; par-outer so both
            # matmuls of a par become ready together (no FIFO head-of-line)
            qt, kt_t = qkT[hp], qkT[CT + hp]
            sTs = {}
            for par in (0, 1):
                sTs[par] = psum_sT.tile([128, 1024], F32, tag="sT",
                                        name=f"sT{hp}_{qc}_{g}_{par}")
            for par in (0, 1):
                ro = par * D
                for j in range(2):
                    kc = 2 * g + j
                    nc.tensor.matmul(
                        sTs[par][:, j * 512:(j + 1) * 512],
                        kt_t[ro:ro + D, kc * 128:(kc + 1) * 128],
                        qt[ro:ro + D, qc * 512:(qc + 1) * 512],
                        start=True, stop=True)
                pexp = expp.tile([128, 1024], BF16, tag="pexp",
                                 name=f"pexp{hp}_{qc}_{g}_{par}")
                nc.scalar.activation(pexp[:], sTs[par][:], Exp, scale=SCALE)
                pexps[par].append(pexp)

        def av_group(hp, avs, pexps, g):
            for j in range(2):
                kc = 2 * g + j
                for par in (0, 1):
                    h = 2 * hp + par
                    nc.tensor.matmul(
                        avs[par][0:E, :],
                        v_aug[kc].rearrange("p (h e) -> p h e", e=E)[:, h, :],
                        pexps[par][g][:, j * 512:(j + 1) * 512],
                        start=(kc == 0), stop=(kc == NT - 1))

        def av_norm(hp, qc, avs):
            for par in (0, 1):
                h, ro, av = 2 * hp + par, par * D, avs[par]
                den = rbp.tile([1, 512], F32, tag="den", name=f"den{h}_{qc}")
                nc.vector.tensor_copy(den[:], av[D:E, :])
                recip = rbp.tile([1, 512], F32, tag="recip",
                                 name=f"rcp{h}_{qc}")
                nc.vector.reciprocal_approx_fast(recip[:], den[:])
                rb = rbp.tile([64, 512], F32, tag="rb", name=f"rb{h}_{qc}")
                nc.gpsimd.partition_broadcast(rb[:], recip[:])
                nc.vector.tensor_mul(
                    outT[hp][ro:ro + D, qc * 512:(qc + 1) * 512],
                    av[0:D, :], rb[:])

        def new_avs(hp, qc):
            return {par: psum_av.tile([128, 512], F32, tag="av",
                                      name=f"av{hp}_{qc}_{par}")
                    for par in (0, 1)}

        # ---- schedule: 2-chunk interleaved ladder ----
        # 12 chunks c_i = (hp, qc). Step i emits c_i's second half (g2, g3 +
        # all four AV groups + normalize) interleaved with c_{i+1}'s first
        # half (g0, g1). Scores allocations alternate chunks, so the 2-deep
        # sT ring behaves like a 3-deep single-chain ring (exp spine can run
        # back-to-back) and every chunk boundary is hidden under the other
        # chunk's steady state. qk/v run through the separate 1-bank pool.
        chunks = [(hp, qc) for hp in range(CT) for qc in range(NQC)]
        NCH = len(chunks)
        pexps_c = [{0: [], 1: []} for _ in range(NCH)]
        avs_c = [None] * NCH

        def sc_c(ci, g):
            hp, qc = chunks[ci]
            sc_group(hp, qc, g, pexps_c[ci])

        def av_c(ci, g):
            hp, qc = chunks[ci]
            if avs_c[ci] is None:
                avs_c[ci] = new_avs(hp, qc)
            av_group(hp, avs_c[ci], pexps_c[ci], g)

        def norm_c(ci):
            hp, qc = chunks[ci]
            av_norm(hp, qc, avs_c[ci])

        # per-step filler injections: k/q tile halves for head pair h are
        # spread over steps 2h-3 and 2h-2 (ready well before chunk 2h's g0)
        inj = {}

        def add_inj(step, slot, fn):
            inj.setdefault((step, slot), []).append(fn)

        add_inj(0, 0, lambda: qk_half(CT + 1, 0))
        add_inj(0, 1, lambda: qk_half(CT + 1, 1))
        add_inj(0, 2, lambda: qk_half(1, 0))
        add_inj(0, 3, lambda: qk_half(1, 1))
        for h in range(2, CT):
            add_inj(2 * h - 3, 1, lambda h=h: qk_half(CT + h, 0))
            add_inj(2 * h - 3, 3, lambda h=h: qk_half(CT + h, 1))
            add_inj(2 * h - 2, 1, lambda h=h: qk_half(h, 0))
            add_inj(2 * h - 2, 3, lambda h=h: qk_half(h, 1))

        def fill(step, slot):
            for fn in inj.get((step, slot), ()):
                fn()

        # prologue: first head pair's k/q tiles, ALL of c0's scores (fills
        # the 8-slot pexp ring so the exp spine runs through the v pass)
        qk_tile(CT + 0)
        qk_tile(0)
        for g in range(NG):
            sc_c(0, g)
        for m in range(NT):
            v_half(m, 0)
            v_half(m, 1)

        for i in range(NCH):
            nxt = i + 1 < NCH
            if i > 0:
                sc_c(i, 2)
            fill(i, 0)
            av_c(i, 0)
            if nxt:
                sc_c(i + 1, 0)
            fill(i, 1)
            av_c(i, 1)
            if i > 0:
                sc_c(i, 3)
            fill(i, 2)
            av_c(i, 2)
            if nxt:
                sc_c(i + 1, 1)
            fill(i, 3)
            av_c(i, 3)
            norm_c(i)
            if i == 1:
                # w_proj load deferred out of the bandwidth-critical lead
                for k in range(CT):
                    wpst = stage.tile([128, C], F32, tag="wpst",
                                      name=f"wpst{k}")
                    nc.sync.dma_start(out=wpst[:],
                                      in_=wproj_ext[k * 128:(k + 1) * 128, :])
                    nc.vector.tensor_copy(wp_bf[k][:], wpst[:])

        # ---- output projection ----
        for m in range(NT):
            ysb = stage.tile([128, C], F32, tag="ysb", name=f"ysb{m}", bufs=4)
            if m % 3 == 2:
                yp_a = psum_av.tile([128, 512], F32, tag="av", name=f"yps{m}a")
                yp_b = psum_av.tile([128, 512], F32, tag="av", name=f"yps{m}b")
                halves = {0: yp_a[:, 0:512], 512: yp_b[:, 0:256]}
            else:
                yps = psum_sT.tile([128, 1024], F32, tag="sT", name=f"yps{m}")
                halves = {0: yps[:, 0:512], 512: yps[:, 512:768]}
            for cs, cw in ((0, 512), (512, 256)):
                dst = halves[cs]
                for kt in range(CT):
                    nc.tensor.matmul(dst,
                                     outT[kt][:, m * 128:(m + 1) * 128],
                                     wp_bf[kt][:, cs:cs + cw],
                                     start=(kt == 0), stop=(kt == CT - 1))
                nc.vector.tensor_add(ysb[:, cs:cs + cw], dst,
                                     b_bcast[:, cs:cs + cw])
                nc.sync.dma_start(out=out_ext[m * 128:(m + 1) * 128, cs:cs + cw],
                                  in_=ysb[:, cs:cs + cw])

    nc.finalize()
    return nc


_NC_CACHE = None


def kernel(x, w_qkv, w_proj, b_proj, trace=False, trace_kwargs=None):
    global _NC_CACHE
    _install_axon_profile_hook()
    from concourse.bass_utils import run_bass_kernel_spmd

    if _NC_CACHE is None:
        _NC_CACHE = build_kernel()
    nc = _NC_CACHE

    x = np.asarray(x, dtype=np.float32)
    w_qkv = np.ascontiguousarray(np.asarray(w_qkv, dtype=np.float32))
    w_proj = np.ascontiguousarray(np.asarray(w_proj, dtype=np.float32))
    b_proj = np.ascontiguousarray(np.asarray(b_proj, dtype=np.float32))
    B = x.shape[0]
    in_maps = [{
        "x": np.ascontiguousarray(x[i]),
        "w_qkv": w_qkv,
        "w_proj": w_proj,
        "b_proj": b_proj,
    } for i in range(B)]

    kwargs = {}
    if trace:
        kwargs["trace"] = True
        if trace_kwargs:
            kwargs.update(trace_kwargs)
    res = run_bass_kernel_spmd(nc, in_maps, core_ids=list(range(B)), **kwargs)
    out = np.stack([res.results[i]["out"] for i in range(B)]).astype(np.float32)
    if trace:
        return out, res
    return out

